# revision 17
# baseline (speedup 1.0000x reference)
"""IsoGMM loss kernel for 8 Trainium2 NeuronCores.

loss = mean_{n,k} r[n,k] * ||X[n] - mus[k]||^2

Decomposition (the entire loss folds into ONE accumulated PE matmul per core):
  sum_{n,k} r*d2 = T1 + T2 - 2*T3
    T1 = sum_n xsq_n * R_n        (xsq_n = ||X[n]||^2, R_n = sum_k r[n,k])
    T2 = sum_k musq_k * C_k       (C_k = sum_n r[n,k])
    T3 = sum_{k,d} mus[k,d] * M[k,d],  M = r.T @ X

Host augments X rows to width 132: [X | 1 | xsq*2^-4 | pad pad], all fp8
e4m3 (xsq is computed host-side from the fp32 X, so no on-chip DVE work
at all). r ships as fp8 too. Tolerance is 2e-2; measured fp8 rel err is
~7e-4 (cross/weight terms only pass through the quantized values, musq
stays fp64 on host). Per 128-row segment:
  ps[64,132] += r_seg.T @ [X | 1 | xsq]_seg       (fp8 matmul, fp32 PSUM)
giving cols 0:128 = M, col 128 = C_k, col 129 = 2^-4 * A_k
(A_k = sum_n r[n,k]*xsq_n). Final partial = sum([-2*mus | musq | 16] * ps).

Perf notes (from NTFF traces):
- Each DMA_DIRECT2D doorbell costs ~700 ns *serialized* on its issuing
  queue, so X and r are packed into ONE dram tensor with chunk-major
  layout -> one doorbell per chunk. All input doorbells go on the sync
  queue IN CONSUMPTION ORDER: with two issuing queues the 16 DMA engines
  round-robin between the queue streams and chunk completions arrive
  out of order, starving the PE mid-kernel.
- A tiny warmup DMA on the (otherwise idle) scalar queue rings the DMA
  engines immediately so their ~1.2 us cold-start overlaps the first
  real doorbell; removing it measurably delays the input stream.
- fp8e4 DoubleRow matmuls contract two 128-row segments per instruction
  (0.5 cycles/row), halving PE instruction count (the per-instruction
  issue cost, not the stream time, is what bounds the PE here).
- The accumulation is split into TWO PSUM chains: chain A (all but the
  last chunk) ships its panel while the PE finishes chain B (the small
  last chunk), overlapping the ~1.8 us copy+doorbell+fetch out-chain
  with the PE tail. A's out-DMA issues on sync, B's on scalar, so the
  two descriptor generations don't queue behind each other.

Sharding: data-parallel over N, 16384 rows per core. Each SBUF partition
holds 128 *contiguous* rows (row order is irrelevant for every term), so
every DMA is perfectly contiguous per partition.
"""

import ml_dtypes
import numpy as np

import concourse.mybir as mybir
import concourse.tile as tile
from concourse import bacc
from concourse.bass_utils import run_bass_kernel_spmd

N, K, D = 131072, 64, 128
NCORES = 8
W = D + 4            # augmented row width: 128 data + ones + xsq + 2 pad
NS = N // NCORES     # rows per core
RPP = NS // 128      # rows per SBUF partition (= segments per core)
CHUNK_SEGS = (8, 32, 32, 32, 16, 8)   # segments per pipeline chunk
SPLIT_SEG = RPP - CHUNK_SEGS[-1]   # chain A covers [0, SPLIT), B the rest
WARM_PAIRS = 26      # dummy DoubleRow matmuls to ramp the PE p-state
XSQ_SCALE = 2.0 ** -4  # keep the xsq column small in fp8 (range ~[4,14])

F8 = ml_dtypes.float8_e4m3
BPS = W + K          # bytes per row-segment slot in the packed layout


def build_nc(chunk_segs=CHUNK_SEGS):
    segs = RPP
    assert sum(chunk_segs) == segs
    f32 = mybir.dt.float32
    f8 = mybir.dt.float8e4

    # Bacc (not plain Bass): its compile() splits sync waits to satisfy
    # TRN2's 1-wait-per-instruction limit, which walrus enforces.
    nc = bacc.Bacc("TRN2", target_bir_lowering=False, debug=False)
    xr = nc.dram_tensor("xr", [128, segs * BPS], f8, kind="ExternalInput")
    out = nc.dram_tensor("out", [K, 2 * W], f32, kind="ExternalOutput")

    with (
        tile.TileContext(nc) as tc,
        tc.tile_pool(name="xb", bufs=len(chunk_segs)) as xpool,
        tc.tile_pool(name="one", bufs=2) as onepool,
        tc.tile_pool(name="wrm", bufs=1) as warmpool,
        tc.tile_pool(name="dum", bufs=1) as dumpool,
        tc.tile_pool(name="ps", bufs=2, space="PSUM") as pspool,
    ):
        psA = pspool.tile([K, W], f32, tag="psA")
        psB = pspool.tile([K, W], f32, tag="psB")
        # The warmup chain reuses psA: its accumulation group closes
        # (stop=True) before the real chain A opens with start=True,
        # which resets the bank, so the garbage never survives.
        psW = psA

        warm = warmpool.tile([128, 4], f8, tag="warm")
        nc.scalar.dma_start(out=warm, in_=xr[:, 0:4])

        # PE p-state warmup: the Tensor engine clocks 0.65/1.2 GHz when
        # cold and only reaches 2.4 GHz after ~3 us of continuous work
        # (measured: early matmul pairs issue at ~110 ns, late ones at
        # ~58 ns). Run dummy DoubleRow matmuls on a zeroed scratch tile
        # during the preamble/doorbell shadow so the real matmuls start
        # at full clock. psW is never read.
        # uint8 tile (DVE memset doesn't lower for fp8) bitcast to fp8
        # for the matmuls; zero bits == fp8 +0.0.
        dumt = dumpool.tile([128, 2 * BPS], mybir.dt.uint8, tag="dummy")
        nc.vector.memset(dumt, 0)
        dumf = dumt.bitcast(f8)
        # dual-fp8 LDWEIGHTS requires the two k-tiles contiguous
        # (s3_lw_dual_fp8_restrictions): stride over dim1 == free size.
        dum_l = dumf[:, 0:2 * K].rearrange("p (t m) -> p t m", t=2)
        dum_r = dumf[:, 2 * K:2 * K + 2 * W].rearrange("p (t q) -> p t q", t=2)
        for j in range(WARM_PAIRS):
            nc.tensor.matmul(
                psW,
                lhsT=dum_l,
                rhs=dum_r,
                start=(j == 0),
                stop=(j == WARM_PAIRS - 1),
                perf_mode=mybir.MatmulPerfMode.DoubleRow,
            )

        tiles = []
        off = 0
        for c, spc in enumerate(chunk_segs):
            t = xpool.tile([128, spc * BPS], f8, tag="xr")
            nc.sync.dma_start(out=t, in_=xr[:, off:off + spc * BPS])
            tiles.append(t)
            off += spc * BPS

        s = 0
        for c, spc in enumerate(chunk_segs):
            t = tiles[c]
            x3 = t[:, :spc * W].rearrange("p (s w) -> p s w", w=W)
            r3 = t[:, spc * W:].rearrange("p (s k) -> p s k", k=K)
            for j in range(spc // 2):
                ps = psA if s < SPLIT_SEG else psB
                nc.tensor.matmul(
                    ps,
                    lhsT=r3[:, 2 * j:2 * j + 2, :],
                    rhs=x3[:, 2 * j:2 * j + 2, :],
                    start=(s == 0 or s == SPLIT_SEG),
                    stop=(s == SPLIT_SEG - 2 or s == segs - 2),
                    perf_mode=mybir.MatmulPerfMode.DoubleRow,
                )
                s += 2
            if s == SPLIT_SEG:
                # Chain A complete: ship its panel now, overlapping the
                # copy+doorbell+descriptor-fetch latency with chain B's
                # matmuls and the epilogue of the input stream.
                osbA = onepool.tile([K, W], f32, tag="osbA")
                nc.vector.tensor_copy(osbA, psA)
                nc.sync.dma_start(out=out[:, 0:W], in_=osbA)

        osbB = onepool.tile([K, W], f32, tag="osbB")
        nc.vector.tensor_copy(osbB, psB)
        nc.scalar.dma_start(out=out[:, W:2 * W], in_=osbB)

    nc.compile()
    return nc


def make_in_maps(X, r, mus, ncores=NCORES, chunk_segs=CHUNK_SEGS):
    X = np.ascontiguousarray(np.asarray(X, dtype=np.float32))
    r = np.ascontiguousarray(np.asarray(r, dtype=np.float32))
    n = X.shape[0]
    ns = n // ncores

    # Host-side row norms from the full-precision X (the only biased term
    # if it were computed from quantized X), then quantize everything.
    xsq = np.einsum("nd,nd->n", X, X, dtype=np.float32)
    Xa = np.zeros((n, W), F8)
    Xa[:, :D] = X.astype(F8)
    Xa[:, D] = F8(1.0)
    Xa[:, D + 1] = (xsq * XSQ_SCALE).astype(F8)
    r8 = r.astype(F8)

    in_maps = []
    for i in range(ncores):
        x4 = Xa[i * ns:(i + 1) * ns].reshape(128, RPP, W)
        r4 = r8[i * ns:(i + 1) * ns].reshape(128, RPP, K)
        blocks = []
        s = 0
        for spc in chunk_segs:
            blocks.append(x4[:, s:s + spc].reshape(128, spc * W))
            blocks.append(r4[:, s:s + spc].reshape(128, spc * K))
            s += spc
        in_maps.append({"xr": np.ascontiguousarray(np.concatenate(blocks, axis=1))})
    return in_maps


def combine_outputs(results, mus):
    """Unshard: weighted sum of each core's two [K, W] panels -> mean."""
    mus = np.asarray(mus, dtype=np.float32)
    musq = (mus.astype(np.float64) ** 2).sum(1)
    ma = np.concatenate(
        [
            -2.0 * mus.astype(np.float64),
            musq[:, None],
            np.full((K, 1), 1.0 / XSQ_SCALE),
            np.zeros((K, 2)),
        ],
        axis=1,
    )
    total = 0.0
    for res in results:
        o = res["out"].astype(np.float64)
        total += float((ma * (o[:, :W] + o[:, W:])).sum())
    return np.array(total / (N * K), dtype=np.float32)


def kernel(X, r, mus):
    nc = build_nc()
    in_maps = make_in_maps(X, r, mus)
    res = run_bass_kernel_spmd(nc, in_maps, list(range(NCORES)))
    return combine_outputs(res.results[:NCORES], mus)
